# revision 15
# baseline (speedup 1.0000x reference)
"""GCGRU cell (graph-conv GRU, diffusion-conv gates) on 8 TRN2 NeuronCores.

Math (per batch b, N=1024 nodes, D=2 in-feats, U=64 units, S=2 supports):
  x0   = [H_b | inputs_b]                          (N, 66)  (feature-permuted)
  for gate g in {r, u, c}:
    pre_g = x0g @ Wg_m0 + sum_s A_s @ (x0g @ Wg_{m=s+1}) + bias_g
  (reassociated: (A_s @ x0) @ W == A_s @ (x0 @ W), so the N x N supports
   multiply a tiny (N, 64) matrix instead of the other association order)
  r, u = sigmoid(pre_r), sigmoid(pre_u); c = tanh(pre_c with x0c=[r*H|inputs])
  h = u * H + (1 - u) * c

Implementation notes:
  - Data parallel over batch: 32 batches -> 4 per core, no collectives.
  - supports[b] is cast f32->bf16 *during* the HBM->SBUF DMA (SWDGE cast,
    ~0.86x plain-DMA rate) in natural layout (i on partitions, (j,s) free).
  - The j-contraction needs j on partitions, so A is transposed 128x128
    tile-wise on the TensorEngine (transpose-mode matmul with a bf16
    identity), de-interleaving s via a stride-2 free AP.  Four transposed
    tiles share one [128,512] bf16 PSUM tile (one accumulation group) so
    the VectorEngine drains them in one op.
  - Gate pre-activations accumulate in f32 PSUM with the transposed A_s
    tiles as the moving operand; sigmoid/tanh run on the ScalarEngine
    straight out of PSUM; small drains go to ScalarE, gate elementwise to
    GpSimd, keeping the VectorEngine for the big transpose drains.
"""

import numpy as np

import concourse.bacc as bacc
import concourse.mybir as mybir
import concourse.tile as tile
from concourse.bass_utils import run_bass_kernel_spmd
from concourse.masks import make_identity

B, N, D, U, S = 32, 1024, 2, 64, 2
F = D + U                      # 66
NCORES = 8
BPC = B // NCORES              # 4 batches per core
P = 128                        # partitions
JB = N // P                    # 8 j-blocks per support
F32 = mybir.dt.float32
BF16 = mybir.dt.bfloat16

_COMPILED = {}


def _build():
    nc = bacc.Bacc("TRN2", target_bir_lowering=False, debug=False)

    t_inputs = nc.dram_tensor("inputs", [BPC, N, D], F32, kind="ExternalInput")
    t_supports = nc.dram_tensor("supports", [BPC, N, N, S], F32, kind="ExternalInput")
    t_hprev = nc.dram_tensor("h_prev", [BPC, N * U], F32, kind="ExternalInput")
    t_wk = {g: nc.dram_tensor(f"{g}_kernel", [F * 3, U], F32, kind="ExternalInput")
            for g in "ruc"}
    t_wb = {g: nc.dram_tensor(f"{g}_bias", [U], F32, kind="ExternalInput")
            for g in "ruc"}
    t_out = nc.dram_tensor("out", [BPC, N * U], F32, kind="ExternalOutput")

    QC = 1                 # i-tiles per load chunk
    NCH = N // (QC * P)    # 2 chunks per batch

    with tile.TileContext(nc) as tc:
        with (
            tc.tile_pool(name="const", bufs=1) as constp,
            tc.tile_pool(name="wt", bufs=1) as wtp,
            tc.tile_pool(name="abf", bufs=10) as abfp,
            tc.tile_pool(name="at", bufs=2) as atp,
            tc.tile_pool(name="act", bufs=2) as actp,
            tc.tile_pool(name="psA", bufs=4, space="PSUM") as psA,
            tc.tile_pool(name="psB", bufs=3, space="PSUM") as psB,
        ):
            # ---- constants ----
            id_bf = constp.tile([P, P], BF16, tag="id_bf")
            make_identity(nc, id_bf[:])
            id_f32 = constp.tile([P, P], F32, tag="id_f32")
            make_identity(nc, id_f32[:])

            # ---- gate weights, hop blocks, permuted to [H|inputs], bf16 ----
            # W rows are (f, m) pairs, m fastest: row f*3 + m.  One staging
            # DMA per gate (rows permuted to [H|inputs]), bf16 casts on DVE
            # so the GpSimd stream holds nothing but the supports loads.
            wst = {}
            for g in "ruc":
                st = wtp.tile([F, 3 * U], F32, tag=f"wst_{g}", name=f"wst_{g}")
                src = t_wk[g].ap().rearrange("(f three) u -> f (three u)", three=3)
                nc.sync.dma_start(st[0:U, :], src[D:F, :])
                nc.sync.dma_start(st[U:F, :], src[0:D, :])
                wst[g] = st

            def w_block(g, m):
                return wst[g][:, m * U:(m + 1) * U]

            w0ru = wtp.tile([F, 2 * U], BF16, tag="w0ru")
            nc.vector.tensor_copy(w0ru[:, 0:U], w_block("r", 0))
            nc.vector.tensor_copy(w0ru[:, U:2 * U], w_block("u", 0))
            wru_s = []
            for s in range(S):
                w = wtp.tile([F, 2 * U], BF16, tag=f"wru{s}")
                nc.vector.tensor_copy(w[:, 0:U], w_block("r", s + 1))
                nc.vector.tensor_copy(w[:, U:2 * U], w_block("u", s + 1))
                wru_s.append(w)
            wc0 = wtp.tile([F, U], BF16, tag="wc0")
            nc.vector.tensor_copy(wc0[:], w_block("c", 0))
            wc_s = []
            for s in range(S):
                w = wtp.tile([F, U], BF16, tag=f"wcs{s}")
                nc.vector.tensor_copy(w[:], w_block("c", s + 1))
                wc_s.append(w)

            bias = {}
            for g in "ruc":
                bt = wtp.tile([U, 1], F32, tag=f"bias_{g}")
                nc.sync.dma_start(bt[:], t_wb[g].ap().rearrange("(u one) -> u one", one=1))
                bias[g] = bt

            sup4 = t_supports.ap().rearrange(
                "b (q p) j two -> b p q (j two)", p=P)

            for b in range(BPC):
                at = [atp.tile([P, JB * N], BF16, tag=f"at{s}", name=f"at{s}")
                      for s in range(S)]
                abts = []
                for ch in range(NCH):
                    ab = abfp.tile([P, QC * N * S], BF16, tag="abf", name="ab")
                    nc.gpsimd.dma_start(
                        ab[:], sup4[b, :, ch * QC:(ch + 1) * QC, :])
                    abts.append(ab)

                # ---- x0 = [H | inputs], transposed (x0T) in f32 + bf16 ----
                x0n = actp.tile([P, JB * F], F32, tag="x0n")
                nc.sync.dma_start(
                    x0n[:].rearrange("p (jb f) -> p jb f", f=F)[:, :, 0:U],
                    t_hprev.ap()[b].rearrange("(jb p u) -> p jb u", p=P, u=U))
                nc.sync.dma_start(
                    x0n[:].rearrange("p (jb f) -> p jb f", f=F)[:, :, U:F],
                    t_inputs.ap()[b].rearrange("(jb p) d -> p jb d", p=P))
                x0T = actp.tile([F, N], F32, tag="x0T")
                x0Tb = actp.tile([F, N], BF16, tag="x0Tb")
                for jb2 in range(JB // 2):
                    px = psA.tile([F, 2 * P], F32, tag="psAx", bufs=2)
                    for q in range(2):
                        nc.tensor.matmul(
                            px[:, q * P:(q + 1) * P],
                            x0n[:, (2 * jb2 + q) * F:(2 * jb2 + q + 1) * F],
                            id_f32[:], start=(q == 0), stop=(q == 1),
                            is_transpose=True)
                    nc.scalar.copy(x0T[:, jb2 * 2 * P:(jb2 + 1) * 2 * P], px[:])
                    nc.scalar.copy(x0Tb[:, jb2 * 2 * P:(jb2 + 1) * 2 * P], px[:])

                # ---- Z_ru_s = x0 @ [Wr_{s+1} | Wu_{s+1}]  (N, 128) bf16 ----
                zru = []
                for s in range(S):
                    z = actp.tile([P, JB * 2 * U], BF16, tag=f"zru{s}")
                    for jb2 in range(JB // 2):
                        pz = psA.tile([P, 2 * 2 * U], F32, tag="psAx", bufs=2)
                        for q in range(2):
                            nc.tensor.matmul(
                                pz[:, q * 2 * U:(q + 1) * 2 * U],
                                x0Tb[:, (2 * jb2 + q) * P:(2 * jb2 + q + 1) * P],
                                wru_s[s][:], start=(q == 0), stop=(q == 1))
                        nc.scalar.copy(
                            z[:, jb2 * 4 * U:(jb2 + 1) * 4 * U], pz[:])
                    zru.append(z)

                # ---- supports loads (cast f32->bf16 in-DMA) + transposes,
                #      interleaved with the phase-1 halves so the PE always
                #      has ready work during the 8 MB load window ----
                def load_and_transpose(ch):
                    ab = abts[ch]
                    ab4 = ab[:].rearrange("p (q j two) -> p q j two", q=QC, two=2)
                    for s in range(S):
                        for jb in range(JB):
                            pt4 = psA.tile([P, QC * P], F32, tag="psA",
                                           bufs=3, name="pt4")
                            for q in range(QC):
                                nc.tensor.matmul(
                                    pt4[:, q * P:(q + 1) * P],
                                    ab4[:, q, jb * P:(jb + 1) * P, s],
                                    id_bf[:],
                                    start=(q == 0), stop=(q == QC - 1))
                            nc.vector.tensor_copy(
                                at[s][:, jb * N + ch * QC * P:
                                      jb * N + (ch + 1) * QC * P],
                                pt4[:])

                NC2 = N // 2
                rT = actp.tile([U, N], BF16, tag="rT")
                uT = actp.tile([U, N], F32, tag="uT")

                NQ = N // NCH

                def phase1(ic):
                    # quarter-column group: only needs chunk ic's transposes
                    p1 = psB.tile([P, NQ], F32, tag="psB", name="p1")
                    k = 0
                    for s in range(S):
                        for jb in range(JB):
                            nc.tensor.matmul(
                                p1[:],
                                zru[s][:, jb * 2 * U:(jb + 1) * 2 * U],
                                at[s][:, jb * N + ic * NQ: jb * N + (ic + 1) * NQ],
                                start=(k == 0), stop=False)
                            k += 1
                    nc.tensor.matmul(
                        p1[:], w0ru[:], x0Tb[:, ic * NQ:(ic + 1) * NQ],
                        start=False, stop=True)
                    nc.scalar.activation(
                        rT[:, ic * NQ:(ic + 1) * NQ], p1[0:U, :],
                        mybir.ActivationFunctionType.Sigmoid, bias=bias["r"][:])
                    nc.scalar.activation(
                        uT[:, ic * NQ:(ic + 1) * NQ], p1[U:2 * U, :],
                        mybir.ActivationFunctionType.Sigmoid, bias=bias["u"][:])

                for ch in range(NCH):
                    load_and_transpose(ch)
                    phase1(ch)

                # ---- x0c^T = [(r * H)^T | inputs^T] (bf16), col-chunked so
                #      the Z_c matmuls can start on the first chunk ----
                x0cT = actp.tile([F, N], BF16, tag="x0cT")
                nc.vector.tensor_copy(x0cT[U:F, :], x0Tb[U:F, :])
                for jb in range(JB):
                    nc.vector.tensor_mul(
                        x0cT[0:U, jb * P:(jb + 1) * P],
                        rT[:, jb * P:(jb + 1) * P],
                        x0T[0:U, jb * P:(jb + 1) * P])

                # ---- Z_c_s = x0c @ Wc_{s+1}  (N, 64) bf16 ----
                zc = []
                for s in range(S):
                    z = actp.tile([P, JB * U], BF16, tag=f"zc{s}")
                    for jb2 in range(JB // 2):
                        pz = psA.tile([P, 2 * U], F32, tag="psAx", bufs=2)
                        for q in range(2):
                            nc.tensor.matmul(
                                pz[:, q * U:(q + 1) * U],
                                x0cT[:, (2 * jb2 + q) * P:(2 * jb2 + q + 1) * P],
                                wc_s[s][:], start=(q == 0), stop=(q == 1))
                        nc.scalar.copy(
                            z[:, jb2 * 2 * U:(jb2 + 1) * 2 * U], pz[:])
                    zc.append(z)

                # ---- phase 2: pre_c^T ----
                cT = actp.tile([U, N], F32, tag="cT")
                for ic in range(NCH):
                    p2 = psB.tile([U, NQ], F32, tag="psB", name="p2")
                    k = 0
                    for s in range(S):
                        for jb in range(JB):
                            nc.tensor.matmul(
                                p2[:],
                                zc[s][:, jb * U:(jb + 1) * U],
                                at[s][:, jb * N + ic * NQ: jb * N + (ic + 1) * NQ],
                                start=(k == 0), stop=False)
                            k += 1
                    nc.tensor.matmul(
                        p2[:], wc0[:], x0cT[:, ic * NQ:(ic + 1) * NQ],
                        start=False, stop=True)
                    nc.scalar.activation(
                        cT[:, ic * NQ:(ic + 1) * NQ], p2[:],
                        mybir.ActivationFunctionType.Tanh, bias=bias["c"][:])

                # ---- h^T = c^T + u^T * (H^T - c^T);  back to natural ----
                hT = actp.tile([U, N], F32, tag="hT")
                nc.vector.tensor_sub(hT[:], x0T[0:U, :], cT[:])
                nc.vector.tensor_mul(hT[:], hT[:], uT[:])
                nc.vector.tensor_add(hT[:], hT[:], cT[:])
                hnat = actp.tile([P, JB * U], F32, tag="hnat")
                for jb2 in range(JB // 2):
                    ph = psA.tile([P, 2 * U], F32, tag="psAx", bufs=2)
                    for q in range(2):
                        nc.tensor.matmul(
                            ph[:, q * U:(q + 1) * U],
                            hT[:, (2 * jb2 + q) * P:(2 * jb2 + q + 1) * P],
                            id_f32[0:U, 0:U], start=(q == 0), stop=(q == 1),
                            is_transpose=True)
                    nc.scalar.copy(hnat[:, jb2 * 2 * U:(jb2 + 1) * 2 * U], ph[:])
                nc.sync.dma_start(
                    t_out.ap()[b].rearrange("(jb p u) -> p jb u", p=P, u=U),
                    hnat[:].rearrange("p (jb u) -> p jb u", u=U))

    nc.finalize()
    return nc


def _make_in_maps(inputs):
    in_maps = []
    for c in range(NCORES):
        lo, hi = c * BPC, (c + 1) * BPC
        in_maps.append({
            "inputs": np.ascontiguousarray(inputs["inputs"][lo:hi], np.float32),
            "supports": np.ascontiguousarray(inputs["supports"][lo:hi], np.float32),
            "h_prev": np.ascontiguousarray(inputs["h_prev"][lo:hi], np.float32),
            "r_kernel": np.ascontiguousarray(inputs["r_kernel"], np.float32),
            "u_kernel": np.ascontiguousarray(inputs["u_kernel"], np.float32),
            "c_kernel": np.ascontiguousarray(inputs["c_kernel"], np.float32),
            "r_bias": np.ascontiguousarray(inputs["r_bias"], np.float32),
            "u_bias": np.ascontiguousarray(inputs["u_bias"], np.float32),
            "c_bias": np.ascontiguousarray(inputs["c_bias"], np.float32),
        })
    return in_maps


def kernel(**inputs):
    nc = _COMPILED.get("nc")
    if nc is None:
        nc = _COMPILED["nc"] = _build()

    res = run_bass_kernel_spmd(nc, _make_in_maps(inputs), core_ids=list(range(NCORES)))
    out = np.concatenate([res.results[c]["out"] for c in range(NCORES)], axis=0)
    return out.astype(np.float32)


# revision 16
# speedup vs baseline: 1.1464x; 1.1464x over previous
"""GCGRU cell (graph-conv GRU, diffusion-conv gates) on 8 TRN2 NeuronCores.

Math (per batch b, N=1024 nodes, D=2 in-feats, U=64 units, S=2 supports):
  x0   = [H_b | inputs_b]                          (N, 66)  (feature-permuted)
  for gate g in {r, u, c}:
    pre_g = x0g @ Wg_m0 + sum_s A_s @ (x0g @ Wg_{m=s+1}) + bias_g
  (reassociated: (A_s @ x0) @ W == A_s @ (x0 @ W), so the N x N supports
   multiply a tiny (N, 64) matrix instead of the other association order)
  r, u = sigmoid(pre_r), sigmoid(pre_u); c = tanh(pre_c with x0c=[r*H|inputs])
  h = u * H + (1 - u) * c

Implementation notes:
  - Data parallel over batch: 32 batches -> 4 per core, no collectives.
  - supports[b] is cast f32->bf16 *during* the HBM->SBUF DMA (SWDGE cast,
    ~0.86x plain-DMA rate) in natural layout (i on partitions, (j,s) free).
  - The j-contraction needs j on partitions, so A is transposed 128x128
    tile-wise on the TensorEngine (transpose-mode matmul with a bf16
    identity), de-interleaving s via a stride-2 free AP.  Four transposed
    tiles share one [128,512] bf16 PSUM tile (one accumulation group) so
    the VectorEngine drains them in one op.
  - Gate pre-activations accumulate in f32 PSUM with the transposed A_s
    tiles as the moving operand; sigmoid/tanh run on the ScalarEngine
    straight out of PSUM; small drains go to ScalarE, gate elementwise to
    GpSimd, keeping the VectorEngine for the big transpose drains.
"""

import numpy as np

import concourse.bacc as bacc
import concourse.mybir as mybir
import concourse.tile as tile
from concourse.bass_utils import run_bass_kernel_spmd
from concourse.masks import make_identity

B, N, D, U, S = 32, 1024, 2, 64, 2
F = D + U                      # 66
NCORES = 8
BPC = B // NCORES              # 4 batches per core
P = 128                        # partitions
JB = N // P                    # 8 j-blocks per support
F32 = mybir.dt.float32
BF16 = mybir.dt.bfloat16

_COMPILED = {}


def _build():
    nc = bacc.Bacc("TRN2", target_bir_lowering=False, debug=False)

    t_inputs = nc.dram_tensor("inputs", [BPC, N, D], F32, kind="ExternalInput")
    t_supports = nc.dram_tensor("supports", [BPC, N, N, S], F32, kind="ExternalInput")
    t_hprev = nc.dram_tensor("h_prev", [BPC, N * U], F32, kind="ExternalInput")
    t_wk = {g: nc.dram_tensor(f"{g}_kernel", [F * 3, U], F32, kind="ExternalInput")
            for g in "ruc"}
    t_wb = {g: nc.dram_tensor(f"{g}_bias", [U], F32, kind="ExternalInput")
            for g in "ruc"}
    t_out = nc.dram_tensor("out", [BPC, N * U], F32, kind="ExternalOutput")

    QC = 2                 # i-tiles per load chunk
    NCH = N // (QC * P)    # 2 chunks per batch

    with tile.TileContext(nc) as tc:
        with (
            tc.tile_pool(name="const", bufs=1) as constp,
            tc.tile_pool(name="wt", bufs=1) as wtp,
            tc.tile_pool(name="abf", bufs=6) as abfp,
            tc.tile_pool(name="at", bufs=2) as atp,
            tc.tile_pool(name="act", bufs=2) as actp,
            tc.tile_pool(name="psA", bufs=4, space="PSUM") as psA,
            tc.tile_pool(name="psB", bufs=3, space="PSUM") as psB,
        ):
            # ---- constants ----
            id_bf = constp.tile([P, P], BF16, tag="id_bf")
            make_identity(nc, id_bf[:])
            id_f32 = constp.tile([P, P], F32, tag="id_f32")
            make_identity(nc, id_f32[:])

            # ---- gate weights, hop blocks, permuted to [H|inputs], bf16 ----
            # W rows are (f, m) pairs, m fastest: row f*3 + m.  One staging
            # DMA per gate (rows permuted to [H|inputs]), bf16 casts on DVE
            # so the GpSimd stream holds nothing but the supports loads.
            wst = {}
            for g in "ruc":
                st = wtp.tile([F, 3 * U], F32, tag=f"wst_{g}", name=f"wst_{g}")
                src = t_wk[g].ap().rearrange("(f three) u -> f (three u)", three=3)
                nc.sync.dma_start(st[0:U, :], src[D:F, :])
                nc.sync.dma_start(st[U:F, :], src[0:D, :])
                wst[g] = st

            def w_block(g, m):
                return wst[g][:, m * U:(m + 1) * U]

            w0ru = wtp.tile([F, 2 * U], BF16, tag="w0ru")
            nc.vector.tensor_copy(w0ru[:, 0:U], w_block("r", 0))
            nc.vector.tensor_copy(w0ru[:, U:2 * U], w_block("u", 0))
            wru_s = []
            for s in range(S):
                w = wtp.tile([F, 2 * U], BF16, tag=f"wru{s}")
                nc.vector.tensor_copy(w[:, 0:U], w_block("r", s + 1))
                nc.vector.tensor_copy(w[:, U:2 * U], w_block("u", s + 1))
                wru_s.append(w)
            wc0 = wtp.tile([F, U], BF16, tag="wc0")
            nc.vector.tensor_copy(wc0[:], w_block("c", 0))
            wc_s = []
            for s in range(S):
                w = wtp.tile([F, U], BF16, tag=f"wcs{s}")
                nc.vector.tensor_copy(w[:], w_block("c", s + 1))
                wc_s.append(w)

            bias = {}
            for g in "ruc":
                bt = wtp.tile([U, 1], F32, tag=f"bias_{g}")
                nc.sync.dma_start(bt[:], t_wb[g].ap().rearrange("(u one) -> u one", one=1))
                bias[g] = bt

            sup4 = t_supports.ap().rearrange(
                "b (q p) j two -> b p q (j two)", p=P)

            for b in range(BPC):
                at = [atp.tile([P, JB * N], BF16, tag=f"at{s}", name=f"at{s}")
                      for s in range(S)]
                abts = []
                for ch in range(NCH):
                    ab = abfp.tile([P, QC * N * S], BF16, tag="abf", name="ab")
                    nc.gpsimd.dma_start(
                        ab[:], sup4[b, :, ch * QC:(ch + 1) * QC, :])
                    abts.append(ab)

                # ---- x0 = [H | inputs], transposed (x0T) in f32 + bf16 ----
                x0n = actp.tile([P, JB * F], F32, tag="x0n")
                nc.sync.dma_start(
                    x0n[:].rearrange("p (jb f) -> p jb f", f=F)[:, :, 0:U],
                    t_hprev.ap()[b].rearrange("(jb p u) -> p jb u", p=P, u=U))
                nc.sync.dma_start(
                    x0n[:].rearrange("p (jb f) -> p jb f", f=F)[:, :, U:F],
                    t_inputs.ap()[b].rearrange("(jb p) d -> p jb d", p=P))
                x0T = actp.tile([F, N], F32, tag="x0T")
                x0Tb = actp.tile([F, N], BF16, tag="x0Tb")
                for jb2 in range(JB // 2):
                    px = psA.tile([F, 2 * P], F32, tag="psAx", bufs=2)
                    for q in range(2):
                        nc.tensor.matmul(
                            px[:, q * P:(q + 1) * P],
                            x0n[:, (2 * jb2 + q) * F:(2 * jb2 + q + 1) * F],
                            id_f32[:], start=(q == 0), stop=(q == 1),
                            is_transpose=True)
                    nc.scalar.copy(x0T[:, jb2 * 2 * P:(jb2 + 1) * 2 * P], px[:])
                    nc.scalar.copy(x0Tb[:, jb2 * 2 * P:(jb2 + 1) * 2 * P], px[:])

                # ---- Z_ru_s = x0 @ [Wr_{s+1} | Wu_{s+1}]  (N, 128) bf16 ----
                zru = []
                for s in range(S):
                    z = actp.tile([P, JB * 2 * U], BF16, tag=f"zru{s}")
                    for jb2 in range(JB // 2):
                        pz = psA.tile([P, 2 * 2 * U], F32, tag="psAx", bufs=2)
                        for q in range(2):
                            nc.tensor.matmul(
                                pz[:, q * 2 * U:(q + 1) * 2 * U],
                                x0Tb[:, (2 * jb2 + q) * P:(2 * jb2 + q + 1) * P],
                                wru_s[s][:], start=(q == 0), stop=(q == 1))
                        nc.scalar.copy(
                            z[:, jb2 * 4 * U:(jb2 + 1) * 4 * U], pz[:])
                    zru.append(z)

                # ---- supports loads (cast f32->bf16 in-DMA) + transposes,
                #      interleaved with the phase-1 halves so the PE always
                #      has ready work during the 8 MB load window ----
                def load_and_transpose(ch):
                    ab = abts[ch]
                    ab4 = ab[:].rearrange("p (q j two) -> p q j two", q=QC, two=2)
                    for s in range(S):
                        for jb in range(JB):
                            pt4 = psA.tile([P, QC * P], F32, tag="psA",
                                           bufs=3, name="pt4")
                            for q in range(QC):
                                nc.tensor.matmul(
                                    pt4[:, q * P:(q + 1) * P],
                                    ab4[:, q, jb * P:(jb + 1) * P, s],
                                    id_bf[:],
                                    start=(q == 0), stop=(q == QC - 1))
                            nc.vector.tensor_copy(
                                at[s][:, jb * N + ch * QC * P:
                                      jb * N + (ch + 1) * QC * P],
                                pt4[:])

                NC2 = N // 2
                rT = actp.tile([U, N], BF16, tag="rT")
                uT = actp.tile([U, N], F32, tag="uT")

                NQ = N // NCH

                def phase1(ic):
                    # quarter-column group: only needs chunk ic's transposes
                    p1 = psB.tile([P, NQ], F32, tag="psB", name="p1")
                    k = 0
                    for s in range(S):
                        for jb in range(JB):
                            nc.tensor.matmul(
                                p1[:],
                                zru[s][:, jb * 2 * U:(jb + 1) * 2 * U],
                                at[s][:, jb * N + ic * NQ: jb * N + (ic + 1) * NQ],
                                start=(k == 0), stop=False)
                            k += 1
                    nc.tensor.matmul(
                        p1[:], w0ru[:], x0Tb[:, ic * NQ:(ic + 1) * NQ],
                        start=False, stop=True)
                    nc.scalar.activation(
                        rT[:, ic * NQ:(ic + 1) * NQ], p1[0:U, :],
                        mybir.ActivationFunctionType.Sigmoid, bias=bias["r"][:])
                    nc.scalar.activation(
                        uT[:, ic * NQ:(ic + 1) * NQ], p1[U:2 * U, :],
                        mybir.ActivationFunctionType.Sigmoid, bias=bias["u"][:])

                for ch in range(NCH):
                    load_and_transpose(ch)
                    phase1(ch)

                # ---- x0c^T = [(r * H)^T | inputs^T] (bf16), col-chunked so
                #      the Z_c matmuls can start on the first chunk ----
                x0cT = actp.tile([F, N], BF16, tag="x0cT")
                nc.vector.tensor_copy(x0cT[U:F, :], x0Tb[U:F, :])
                for jb in range(JB):
                    nc.vector.tensor_mul(
                        x0cT[0:U, jb * P:(jb + 1) * P],
                        rT[:, jb * P:(jb + 1) * P],
                        x0T[0:U, jb * P:(jb + 1) * P])

                # ---- Z_c_s = x0c @ Wc_{s+1}  (N, 64) bf16 ----
                zc = []
                for s in range(S):
                    z = actp.tile([P, JB * U], BF16, tag=f"zc{s}")
                    for jb2 in range(JB // 2):
                        pz = psA.tile([P, 2 * U], F32, tag="psAx", bufs=2)
                        for q in range(2):
                            nc.tensor.matmul(
                                pz[:, q * U:(q + 1) * U],
                                x0cT[:, (2 * jb2 + q) * P:(2 * jb2 + q + 1) * P],
                                wc_s[s][:], start=(q == 0), stop=(q == 1))
                        nc.scalar.copy(
                            z[:, jb2 * 2 * U:(jb2 + 1) * 2 * U], pz[:])
                    zc.append(z)

                # ---- phase 2: pre_c^T ----
                cT = actp.tile([U, N], F32, tag="cT")
                for ic in range(NCH):
                    p2 = psB.tile([U, NQ], F32, tag="psB", name="p2")
                    k = 0
                    for s in range(S):
                        for jb in range(JB):
                            nc.tensor.matmul(
                                p2[:],
                                zc[s][:, jb * U:(jb + 1) * U],
                                at[s][:, jb * N + ic * NQ: jb * N + (ic + 1) * NQ],
                                start=(k == 0), stop=False)
                            k += 1
                    nc.tensor.matmul(
                        p2[:], wc0[:], x0cT[:, ic * NQ:(ic + 1) * NQ],
                        start=False, stop=True)
                    nc.scalar.activation(
                        cT[:, ic * NQ:(ic + 1) * NQ], p2[:],
                        mybir.ActivationFunctionType.Tanh, bias=bias["c"][:])

                # ---- h^T = c^T + u^T * (H^T - c^T);  back to natural ----
                hT = actp.tile([U, N], F32, tag="hT")
                nc.vector.tensor_sub(hT[:], x0T[0:U, :], cT[:])
                nc.vector.tensor_mul(hT[:], hT[:], uT[:])
                nc.vector.tensor_add(hT[:], hT[:], cT[:])
                hnat = actp.tile([P, JB * U], F32, tag="hnat")
                for jb2 in range(JB // 2):
                    ph = psA.tile([P, 2 * U], F32, tag="psAx", bufs=2)
                    for q in range(2):
                        nc.tensor.matmul(
                            ph[:, q * U:(q + 1) * U],
                            hT[:, (2 * jb2 + q) * P:(2 * jb2 + q + 1) * P],
                            id_f32[0:U, 0:U], start=(q == 0), stop=(q == 1),
                            is_transpose=True)
                    nc.scalar.copy(hnat[:, jb2 * 2 * U:(jb2 + 1) * 2 * U], ph[:])
                nc.sync.dma_start(
                    t_out.ap()[b].rearrange("(jb p u) -> p jb u", p=P, u=U),
                    hnat[:].rearrange("p (jb u) -> p jb u", u=U))

    nc.finalize()
    return nc


def _make_in_maps(inputs):
    in_maps = []
    for c in range(NCORES):
        lo, hi = c * BPC, (c + 1) * BPC
        in_maps.append({
            "inputs": np.ascontiguousarray(inputs["inputs"][lo:hi], np.float32),
            "supports": np.ascontiguousarray(inputs["supports"][lo:hi], np.float32),
            "h_prev": np.ascontiguousarray(inputs["h_prev"][lo:hi], np.float32),
            "r_kernel": np.ascontiguousarray(inputs["r_kernel"], np.float32),
            "u_kernel": np.ascontiguousarray(inputs["u_kernel"], np.float32),
            "c_kernel": np.ascontiguousarray(inputs["c_kernel"], np.float32),
            "r_bias": np.ascontiguousarray(inputs["r_bias"], np.float32),
            "u_bias": np.ascontiguousarray(inputs["u_bias"], np.float32),
            "c_bias": np.ascontiguousarray(inputs["c_bias"], np.float32),
        })
    return in_maps


def kernel(**inputs):
    nc = _COMPILED.get("nc")
    if nc is None:
        nc = _COMPILED["nc"] = _build()

    res = run_bass_kernel_spmd(nc, _make_in_maps(inputs), core_ids=list(range(NCORES)))
    out = np.concatenate([res.results[c]["out"] for c in range(NCORES)], axis=0)
    return out.astype(np.float32)


# revision 17
# speedup vs baseline: 1.1947x; 1.0422x over previous
"""GCGRU cell (graph-conv GRU, diffusion-conv gates) on 8 TRN2 NeuronCores.

Math (per batch b, N=1024 nodes, D=2 in-feats, U=64 units, S=2 supports):
  x0   = [H_b | inputs_b]                          (N, 66)  (feature-permuted)
  for gate g in {r, u, c}:
    pre_g = x0g @ Wg_m0 + sum_s A_s @ (x0g @ Wg_{m=s+1}) + bias_g
  (reassociated: (A_s @ x0) @ W == A_s @ (x0 @ W), so the N x N supports
   multiply a tiny (N, 64) matrix instead of the other association order)
  r, u = sigmoid(pre_r), sigmoid(pre_u); c = tanh(pre_c with x0c=[r*H|inputs])
  h = u * H + (1 - u) * c

Implementation notes:
  - Data parallel over batch: 32 batches -> 4 per core, no collectives.
  - supports[b] is cast f32->bf16 *during* the HBM->SBUF DMA (SWDGE cast,
    ~0.86x plain-DMA rate) in natural layout (i on partitions, (j,s) free).
  - The j-contraction needs j on partitions, so A is transposed 128x128
    tile-wise on the TensorEngine (transpose-mode matmul with a bf16
    identity), de-interleaving s via a stride-2 free AP.  Four transposed
    tiles share one [128,512] bf16 PSUM tile (one accumulation group) so
    the VectorEngine drains them in one op.
  - Gate pre-activations accumulate in f32 PSUM with the transposed A_s
    tiles as the moving operand; sigmoid/tanh run on the ScalarEngine
    straight out of PSUM; small drains go to ScalarE, gate elementwise to
    GpSimd, keeping the VectorEngine for the big transpose drains.
"""

import numpy as np

import concourse.bacc as bacc
import concourse.mybir as mybir
import concourse.tile as tile
from concourse.bass_utils import run_bass_kernel_spmd
from concourse.masks import make_identity

B, N, D, U, S = 32, 1024, 2, 64, 2
F = D + U                      # 66
NCORES = 8
BPC = B // NCORES              # 4 batches per core
P = 128                        # partitions
JB = N // P                    # 8 j-blocks per support
F32 = mybir.dt.float32
BF16 = mybir.dt.bfloat16

_COMPILED = {}


def _build():
    nc = bacc.Bacc("TRN2", target_bir_lowering=False, debug=False)

    t_inputs = nc.dram_tensor("inputs", [BPC, N, D], F32, kind="ExternalInput")
    t_supports = nc.dram_tensor("supports", [BPC, N, N, S], F32, kind="ExternalInput")
    t_hprev = nc.dram_tensor("h_prev", [BPC, N * U], F32, kind="ExternalInput")
    t_wk = {g: nc.dram_tensor(f"{g}_kernel", [F * 3, U], F32, kind="ExternalInput")
            for g in "ruc"}
    t_wb = {g: nc.dram_tensor(f"{g}_bias", [U], F32, kind="ExternalInput")
            for g in "ruc"}
    t_out = nc.dram_tensor("out", [BPC, N * U], F32, kind="ExternalOutput")

    QC = 2                 # i-tiles per load chunk
    NCH = N // (QC * P)    # 2 chunks per batch

    with tile.TileContext(nc) as tc:
        with (
            tc.tile_pool(name="const", bufs=1) as constp,
            tc.tile_pool(name="wt", bufs=1) as wtp,
            tc.tile_pool(name="abf", bufs=3) as abfp,
            tc.tile_pool(name="at", bufs=2) as atp,
            tc.tile_pool(name="act", bufs=2) as actp,
            tc.tile_pool(name="psA", bufs=4, space="PSUM") as psA,
            tc.tile_pool(name="psB", bufs=3, space="PSUM") as psB,
        ):
            # ---- constants ----
            id_bf = constp.tile([P, P], BF16, tag="id_bf")
            make_identity(nc, id_bf[:])
            id_f32 = constp.tile([P, P], F32, tag="id_f32")
            make_identity(nc, id_f32[:])

            # ---- gate weights, hop blocks, permuted to [H|inputs], bf16 ----
            # W rows are (f, m) pairs, m fastest: row f*3 + m.  One staging
            # DMA per gate (rows permuted to [H|inputs]), bf16 casts on DVE
            # so the GpSimd stream holds nothing but the supports loads.
            wst = {}
            for g in "ruc":
                st = wtp.tile([F, 3 * U], F32, tag=f"wst_{g}", name=f"wst_{g}")
                src = t_wk[g].ap().rearrange("(f three) u -> f (three u)", three=3)
                nc.sync.dma_start(st[0:U, :], src[D:F, :])
                nc.sync.dma_start(st[U:F, :], src[0:D, :])
                wst[g] = st

            def w_block(g, m):
                return wst[g][:, m * U:(m + 1) * U]

            w0ru = wtp.tile([F, 2 * U], BF16, tag="w0ru")
            nc.vector.tensor_copy(w0ru[:, 0:U], w_block("r", 0))
            nc.vector.tensor_copy(w0ru[:, U:2 * U], w_block("u", 0))
            wru_s = []
            for s in range(S):
                w = wtp.tile([F, 2 * U], BF16, tag=f"wru{s}")
                nc.vector.tensor_copy(w[:, 0:U], w_block("r", s + 1))
                nc.vector.tensor_copy(w[:, U:2 * U], w_block("u", s + 1))
                wru_s.append(w)
            wc0 = wtp.tile([F, U], BF16, tag="wc0")
            nc.vector.tensor_copy(wc0[:], w_block("c", 0))
            wc_s = []
            for s in range(S):
                w = wtp.tile([F, U], BF16, tag=f"wcs{s}")
                nc.vector.tensor_copy(w[:], w_block("c", s + 1))
                wc_s.append(w)

            bias = {}
            for g in "ruc":
                bt = wtp.tile([U, 1], F32, tag=f"bias_{g}")
                nc.sync.dma_start(bt[:], t_wb[g].ap().rearrange("(u one) -> u one", one=1))
                bias[g] = bt

            sup4 = t_supports.ap().rearrange(
                "b (q p) j two -> b p q (j two)", p=P)

            for b in range(BPC):
                at = [atp.tile([P, JB * N], BF16, tag=f"at{s}", name=f"at{s}")
                      for s in range(S)]
                abts = []
                for ch in range(NCH):
                    ab = abfp.tile([P, QC * N * S], BF16, tag="abf", name="ab")
                    nc.gpsimd.dma_start(
                        ab[:], sup4[b, :, ch * QC:(ch + 1) * QC, :])
                    abts.append(ab)

                # ---- x0 = [H | inputs], transposed (x0T) in f32 + bf16 ----
                x0n = actp.tile([P, JB * F], F32, tag="x0n")
                nc.sync.dma_start(
                    x0n[:].rearrange("p (jb f) -> p jb f", f=F)[:, :, 0:U],
                    t_hprev.ap()[b].rearrange("(jb p u) -> p jb u", p=P, u=U))
                nc.sync.dma_start(
                    x0n[:].rearrange("p (jb f) -> p jb f", f=F)[:, :, U:F],
                    t_inputs.ap()[b].rearrange("(jb p) d -> p jb d", p=P))
                x0T = actp.tile([F, N], F32, tag="x0T")
                x0Tb = actp.tile([F, N], BF16, tag="x0Tb")
                for jb2 in range(JB // 2):
                    px = psA.tile([F, 2 * P], F32, tag="psAx", bufs=2)
                    for q in range(2):
                        nc.tensor.matmul(
                            px[:, q * P:(q + 1) * P],
                            x0n[:, (2 * jb2 + q) * F:(2 * jb2 + q + 1) * F],
                            id_f32[:], start=(q == 0), stop=(q == 1),
                            is_transpose=True)
                    nc.scalar.copy(x0T[:, jb2 * 2 * P:(jb2 + 1) * 2 * P], px[:])
                    nc.scalar.copy(x0Tb[:, jb2 * 2 * P:(jb2 + 1) * 2 * P], px[:])

                # ---- Z_ru_s = x0 @ [Wr_{s+1} | Wu_{s+1}]  (N, 128) bf16 ----
                zru = []
                for s in range(S):
                    z = actp.tile([P, JB * 2 * U], BF16, tag=f"zru{s}")
                    for jb2 in range(JB // 2):
                        pz = psA.tile([P, 2 * 2 * U], F32, tag="psAx", bufs=2)
                        for q in range(2):
                            nc.tensor.matmul(
                                pz[:, q * 2 * U:(q + 1) * 2 * U],
                                x0Tb[:, (2 * jb2 + q) * P:(2 * jb2 + q + 1) * P],
                                wru_s[s][:], start=(q == 0), stop=(q == 1))
                        nc.scalar.copy(
                            z[:, jb2 * 4 * U:(jb2 + 1) * 4 * U], pz[:])
                    zru.append(z)

                # ---- supports loads (cast f32->bf16 in-DMA) + transposes,
                #      interleaved with the phase-1 halves so the PE always
                #      has ready work during the 8 MB load window ----
                def load_and_transpose(ch):
                    ab = abts[ch]
                    ab4 = ab[:].rearrange("p (q j two) -> p q j two", q=QC, two=2)
                    for s in range(S):
                        for jb in range(JB):
                            pt4 = psA.tile([P, QC * P], F32, tag="psA",
                                           bufs=3, name="pt4")
                            for q in range(QC):
                                nc.tensor.matmul(
                                    pt4[:, q * P:(q + 1) * P],
                                    ab4[:, q, jb * P:(jb + 1) * P, s],
                                    id_bf[:],
                                    start=(q == 0), stop=(q == QC - 1))
                            nc.vector.tensor_copy(
                                at[s][:, jb * N + ch * QC * P:
                                      jb * N + (ch + 1) * QC * P],
                                pt4[:])

                NC2 = N // 2
                rT = actp.tile([U, N], BF16, tag="rT")
                uT = actp.tile([U, N], F32, tag="uT")

                NQ = N // NCH

                def phase1(ic):
                    # quarter-column group: only needs chunk ic's transposes
                    p1 = psB.tile([P, NQ], F32, tag="psB", name="p1")
                    k = 0
                    for s in range(S):
                        for jb in range(JB):
                            nc.tensor.matmul(
                                p1[:],
                                zru[s][:, jb * 2 * U:(jb + 1) * 2 * U],
                                at[s][:, jb * N + ic * NQ: jb * N + (ic + 1) * NQ],
                                start=(k == 0), stop=False)
                            k += 1
                    nc.tensor.matmul(
                        p1[:], w0ru[:], x0Tb[:, ic * NQ:(ic + 1) * NQ],
                        start=False, stop=True)
                    nc.scalar.activation(
                        rT[:, ic * NQ:(ic + 1) * NQ], p1[0:U, :],
                        mybir.ActivationFunctionType.Sigmoid, bias=bias["r"][:])
                    nc.scalar.activation(
                        uT[:, ic * NQ:(ic + 1) * NQ], p1[U:2 * U, :],
                        mybir.ActivationFunctionType.Sigmoid, bias=bias["u"][:])

                for ch in range(NCH):
                    load_and_transpose(ch)
                    phase1(ch)

                # ---- x0c^T = [(r * H)^T | inputs^T] (bf16), col-chunked so
                #      the Z_c matmuls can start on the first chunk ----
                x0cT = actp.tile([F, N], BF16, tag="x0cT")
                nc.vector.tensor_copy(x0cT[U:F, :], x0Tb[U:F, :])
                for jb in range(JB):
                    nc.vector.tensor_mul(
                        x0cT[0:U, jb * P:(jb + 1) * P],
                        rT[:, jb * P:(jb + 1) * P],
                        x0T[0:U, jb * P:(jb + 1) * P])

                # ---- Z_c_s = x0c @ Wc_{s+1}  (N, 64) bf16 ----
                zc = []
                for s in range(S):
                    z = actp.tile([P, JB * U], BF16, tag=f"zc{s}")
                    for jb2 in range(JB // 2):
                        pz = psA.tile([P, 2 * U], F32, tag="psAx", bufs=2)
                        for q in range(2):
                            nc.tensor.matmul(
                                pz[:, q * U:(q + 1) * U],
                                x0cT[:, (2 * jb2 + q) * P:(2 * jb2 + q + 1) * P],
                                wc_s[s][:], start=(q == 0), stop=(q == 1))
                        nc.scalar.copy(
                            z[:, jb2 * 2 * U:(jb2 + 1) * 2 * U], pz[:])
                    zc.append(z)

                # ---- phase 2: pre_c^T ----
                cT = actp.tile([U, N], F32, tag="cT")
                for ic in range(NCH):
                    p2 = psB.tile([U, NQ], F32, tag="psB", name="p2")
                    k = 0
                    for s in range(S):
                        for jb in range(JB):
                            nc.tensor.matmul(
                                p2[:],
                                zc[s][:, jb * U:(jb + 1) * U],
                                at[s][:, jb * N + ic * NQ: jb * N + (ic + 1) * NQ],
                                start=(k == 0), stop=False)
                            k += 1
                    nc.tensor.matmul(
                        p2[:], wc0[:], x0cT[:, ic * NQ:(ic + 1) * NQ],
                        start=False, stop=True)
                    nc.scalar.activation(
                        cT[:, ic * NQ:(ic + 1) * NQ], p2[:],
                        mybir.ActivationFunctionType.Tanh, bias=bias["c"][:])

                # ---- h^T = c^T + u^T * (H^T - c^T);  back to natural ----
                hT = actp.tile([U, N], F32, tag="hT")
                nc.vector.tensor_sub(hT[:], x0T[0:U, :], cT[:])
                nc.vector.tensor_mul(hT[:], hT[:], uT[:])
                nc.vector.tensor_add(hT[:], hT[:], cT[:])
                hnat = actp.tile([P, JB * U], F32, tag="hnat")
                for jb2 in range(JB // 2):
                    ph = psA.tile([P, 2 * U], F32, tag="psAx", bufs=2)
                    for q in range(2):
                        nc.tensor.matmul(
                            ph[:, q * U:(q + 1) * U],
                            hT[:, (2 * jb2 + q) * P:(2 * jb2 + q + 1) * P],
                            id_f32[0:U, 0:U], start=(q == 0), stop=(q == 1),
                            is_transpose=True)
                    nc.scalar.copy(hnat[:, jb2 * 2 * U:(jb2 + 1) * 2 * U], ph[:])
                nc.sync.dma_start(
                    t_out.ap()[b].rearrange("(jb p u) -> p jb u", p=P, u=U),
                    hnat[:].rearrange("p (jb u) -> p jb u", u=U))

    nc.finalize()
    return nc


def _make_in_maps(inputs):
    in_maps = []
    for c in range(NCORES):
        lo, hi = c * BPC, (c + 1) * BPC
        in_maps.append({
            "inputs": np.ascontiguousarray(inputs["inputs"][lo:hi], np.float32),
            "supports": np.ascontiguousarray(inputs["supports"][lo:hi], np.float32),
            "h_prev": np.ascontiguousarray(inputs["h_prev"][lo:hi], np.float32),
            "r_kernel": np.ascontiguousarray(inputs["r_kernel"], np.float32),
            "u_kernel": np.ascontiguousarray(inputs["u_kernel"], np.float32),
            "c_kernel": np.ascontiguousarray(inputs["c_kernel"], np.float32),
            "r_bias": np.ascontiguousarray(inputs["r_bias"], np.float32),
            "u_bias": np.ascontiguousarray(inputs["u_bias"], np.float32),
            "c_bias": np.ascontiguousarray(inputs["c_bias"], np.float32),
        })
    return in_maps


def kernel(**inputs):
    nc = _COMPILED.get("nc")
    if nc is None:
        nc = _COMPILED["nc"] = _build()

    res = run_bass_kernel_spmd(nc, _make_in_maps(inputs), core_ids=list(range(NCORES)))
    out = np.concatenate([res.results[c]["out"] for c in range(NCORES)], axis=0)
    return out.astype(np.float32)


# revision 18
# speedup vs baseline: 1.2007x; 1.0050x over previous
"""GCGRU cell (graph-conv GRU, diffusion-conv gates) on 8 TRN2 NeuronCores.

Math (per batch b, N=1024 nodes, D=2 in-feats, U=64 units, S=2 supports):
  x0   = [H_b | inputs_b]                          (N, 66)  (feature-permuted)
  for gate g in {r, u, c}:
    pre_g = x0g @ Wg_m0 + sum_s A_s @ (x0g @ Wg_{m=s+1}) + bias_g
  (reassociated: (A_s @ x0) @ W == A_s @ (x0 @ W), so the N x N supports
   multiply a tiny (N, 64) matrix instead of the other association order)
  r, u = sigmoid(pre_r), sigmoid(pre_u); c = tanh(pre_c with x0c=[r*H|inputs])
  h = u * H + (1 - u) * c

Implementation notes:
  - Data parallel over batch: 32 batches -> 4 per core, no collectives.
  - supports[b] is cast f32->bf16 *during* the HBM->SBUF DMA (SWDGE cast,
    ~0.86x plain-DMA rate) in natural layout (i on partitions, (j,s) free).
  - The j-contraction needs j on partitions, so A is transposed 128x128
    tile-wise on the TensorEngine (transpose-mode matmul with a bf16
    identity), de-interleaving s via a stride-2 free AP.  Four transposed
    tiles share one [128,512] bf16 PSUM tile (one accumulation group) so
    the VectorEngine drains them in one op.
  - Gate pre-activations accumulate in f32 PSUM with the transposed A_s
    tiles as the moving operand; sigmoid/tanh run on the ScalarEngine
    straight out of PSUM; small drains go to ScalarE, gate elementwise to
    GpSimd, keeping the VectorEngine for the big transpose drains.
"""

import numpy as np

import concourse.bacc as bacc
import concourse.mybir as mybir
import concourse.tile as tile
from concourse.bass_utils import run_bass_kernel_spmd
from concourse.masks import make_identity

B, N, D, U, S = 32, 1024, 2, 64, 2
F = D + U                      # 66
NCORES = 8
BPC = B // NCORES              # 4 batches per core
P = 128                        # partitions
JB = N // P                    # 8 j-blocks per support
F32 = mybir.dt.float32
BF16 = mybir.dt.bfloat16

_COMPILED = {}


def _build():
    nc = bacc.Bacc("TRN2", target_bir_lowering=False, debug=False)

    t_inputs = nc.dram_tensor("inputs", [BPC, N, D], F32, kind="ExternalInput")
    t_supports = nc.dram_tensor("supports", [BPC, N, N, S], F32, kind="ExternalInput")
    t_hprev = nc.dram_tensor("h_prev", [BPC, N * U], F32, kind="ExternalInput")
    t_wk = {g: nc.dram_tensor(f"{g}_kernel", [F * 3, U], F32, kind="ExternalInput")
            for g in "ruc"}
    t_wb = {g: nc.dram_tensor(f"{g}_bias", [U], F32, kind="ExternalInput")
            for g in "ruc"}
    t_out = nc.dram_tensor("out", [BPC, N * U], F32, kind="ExternalOutput")

    QC = 2                 # i-tiles per load chunk
    NCH = N // (QC * P)    # 2 chunks per batch

    with tile.TileContext(nc) as tc:
        with (
            tc.tile_pool(name="const", bufs=1) as constp,
            tc.tile_pool(name="wt", bufs=1) as wtp,
            tc.tile_pool(name="abf", bufs=4) as abfp,
            tc.tile_pool(name="at", bufs=2) as atp,
            tc.tile_pool(name="act", bufs=2) as actp,
            tc.tile_pool(name="psA", bufs=4, space="PSUM") as psA,
            tc.tile_pool(name="psB", bufs=3, space="PSUM") as psB,
        ):
            # ---- constants ----
            id_bf = constp.tile([P, P], BF16, tag="id_bf")
            make_identity(nc, id_bf[:])
            id_f32 = constp.tile([P, P], F32, tag="id_f32")
            make_identity(nc, id_f32[:])

            # ---- gate weights, hop blocks, permuted to [H|inputs], bf16 ----
            # W rows are (f, m) pairs, m fastest: row f*3 + m.  One staging
            # DMA per gate (rows permuted to [H|inputs]), bf16 casts on DVE
            # so the GpSimd stream holds nothing but the supports loads.
            wst = {}
            for g in "ruc":
                st = wtp.tile([F, 3 * U], F32, tag=f"wst_{g}", name=f"wst_{g}")
                src = t_wk[g].ap().rearrange("(f three) u -> f (three u)", three=3)
                nc.sync.dma_start(st[0:U, :], src[D:F, :])
                nc.sync.dma_start(st[U:F, :], src[0:D, :])
                wst[g] = st

            def w_block(g, m):
                return wst[g][:, m * U:(m + 1) * U]

            w0ru = wtp.tile([F, 2 * U], BF16, tag="w0ru")
            nc.vector.tensor_copy(w0ru[:, 0:U], w_block("r", 0))
            nc.vector.tensor_copy(w0ru[:, U:2 * U], w_block("u", 0))
            wru_s = []
            for s in range(S):
                w = wtp.tile([F, 2 * U], BF16, tag=f"wru{s}")
                nc.vector.tensor_copy(w[:, 0:U], w_block("r", s + 1))
                nc.vector.tensor_copy(w[:, U:2 * U], w_block("u", s + 1))
                wru_s.append(w)
            wc0 = wtp.tile([F, U], BF16, tag="wc0")
            nc.vector.tensor_copy(wc0[:], w_block("c", 0))
            wc_s = []
            for s in range(S):
                w = wtp.tile([F, U], BF16, tag=f"wcs{s}")
                nc.vector.tensor_copy(w[:], w_block("c", s + 1))
                wc_s.append(w)

            bias = {}
            for g in "ruc":
                bt = wtp.tile([U, 1], F32, tag=f"bias_{g}")
                nc.sync.dma_start(bt[:], t_wb[g].ap().rearrange("(u one) -> u one", one=1))
                bias[g] = bt

            sup4 = t_supports.ap().rearrange(
                "b (q p) j two -> b p q (j two)", p=P)

            for b in range(BPC):
                at = [atp.tile([P, JB * N], BF16, tag=f"at{s}", name=f"at{s}")
                      for s in range(S)]
                abts = []
                for ch in range(NCH):
                    ab = abfp.tile([P, QC * N * S], BF16, tag="abf", name="ab")
                    nc.gpsimd.dma_start(
                        ab[:], sup4[b, :, ch * QC:(ch + 1) * QC, :])
                    abts.append(ab)

                # ---- x0 = [H | inputs], transposed (x0T) in f32 + bf16 ----
                x0n = actp.tile([P, JB * F], F32, tag="x0n")
                nc.sync.dma_start(
                    x0n[:].rearrange("p (jb f) -> p jb f", f=F)[:, :, 0:U],
                    t_hprev.ap()[b].rearrange("(jb p u) -> p jb u", p=P, u=U))
                nc.sync.dma_start(
                    x0n[:].rearrange("p (jb f) -> p jb f", f=F)[:, :, U:F],
                    t_inputs.ap()[b].rearrange("(jb p) d -> p jb d", p=P))
                x0T = actp.tile([F, N], F32, tag="x0T")
                x0Tb = actp.tile([F, N], BF16, tag="x0Tb")
                for jb2 in range(JB // 2):
                    px = psA.tile([F, 2 * P], F32, tag="psAx", bufs=2)
                    for q in range(2):
                        nc.tensor.matmul(
                            px[:, q * P:(q + 1) * P],
                            x0n[:, (2 * jb2 + q) * F:(2 * jb2 + q + 1) * F],
                            id_f32[:], start=(q == 0), stop=(q == 1),
                            is_transpose=True)
                    nc.scalar.copy(x0T[:, jb2 * 2 * P:(jb2 + 1) * 2 * P], px[:])
                    nc.scalar.copy(x0Tb[:, jb2 * 2 * P:(jb2 + 1) * 2 * P], px[:])

                # ---- Z_ru_s = x0 @ [Wr_{s+1} | Wu_{s+1}]  (N, 128) bf16 ----
                zru = []
                for s in range(S):
                    z = actp.tile([P, JB * 2 * U], BF16, tag=f"zru{s}")
                    for jb2 in range(JB // 2):
                        pz = psA.tile([P, 2 * 2 * U], F32, tag="psAx", bufs=2)
                        for q in range(2):
                            nc.tensor.matmul(
                                pz[:, q * 2 * U:(q + 1) * 2 * U],
                                x0Tb[:, (2 * jb2 + q) * P:(2 * jb2 + q + 1) * P],
                                wru_s[s][:], start=(q == 0), stop=(q == 1))
                        nc.scalar.copy(
                            z[:, jb2 * 4 * U:(jb2 + 1) * 4 * U], pz[:])
                    zru.append(z)

                # ---- supports loads (cast f32->bf16 in-DMA) + transposes,
                #      interleaved with the phase-1 halves so the PE always
                #      has ready work during the 8 MB load window ----
                def load_and_transpose(ch):
                    ab = abts[ch]
                    ab4 = ab[:].rearrange("p (q j two) -> p q j two", q=QC, two=2)
                    for s in range(S):
                        for jb in range(JB):
                            pt4 = psA.tile([P, QC * P], F32, tag="psA",
                                           bufs=3, name="pt4")
                            for q in range(QC):
                                nc.tensor.matmul(
                                    pt4[:, q * P:(q + 1) * P],
                                    ab4[:, q, jb * P:(jb + 1) * P, s],
                                    id_bf[:],
                                    start=(q == 0), stop=(q == QC - 1))
                            nc.vector.tensor_copy(
                                at[s][:, jb * N + ch * QC * P:
                                      jb * N + (ch + 1) * QC * P],
                                pt4[:])

                NC2 = N // 2
                rT = actp.tile([U, N], BF16, tag="rT")
                uT = actp.tile([U, N], F32, tag="uT")

                NQ = N // NCH

                def phase1(ic):
                    # quarter-column group: only needs chunk ic's transposes
                    p1 = psB.tile([P, NQ], F32, tag="psB", name="p1")
                    k = 0
                    for s in range(S):
                        for jb in range(JB):
                            nc.tensor.matmul(
                                p1[:],
                                zru[s][:, jb * 2 * U:(jb + 1) * 2 * U],
                                at[s][:, jb * N + ic * NQ: jb * N + (ic + 1) * NQ],
                                start=(k == 0), stop=False)
                            k += 1
                    nc.tensor.matmul(
                        p1[:], w0ru[:], x0Tb[:, ic * NQ:(ic + 1) * NQ],
                        start=False, stop=True)
                    nc.scalar.activation(
                        rT[:, ic * NQ:(ic + 1) * NQ], p1[0:U, :],
                        mybir.ActivationFunctionType.Sigmoid, bias=bias["r"][:])
                    nc.scalar.activation(
                        uT[:, ic * NQ:(ic + 1) * NQ], p1[U:2 * U, :],
                        mybir.ActivationFunctionType.Sigmoid, bias=bias["u"][:])

                for ch in range(NCH):
                    load_and_transpose(ch)
                    phase1(ch)

                # ---- x0c^T = [(r * H)^T | inputs^T] (bf16), col-chunked so
                #      the Z_c matmuls can start on the first chunk ----
                x0cT = actp.tile([F, N], BF16, tag="x0cT")
                nc.vector.tensor_copy(x0cT[U:F, :], x0Tb[U:F, :])
                for jb in range(JB):
                    nc.vector.tensor_mul(
                        x0cT[0:U, jb * P:(jb + 1) * P],
                        rT[:, jb * P:(jb + 1) * P],
                        x0T[0:U, jb * P:(jb + 1) * P])

                # ---- Z_c_s = x0c @ Wc_{s+1}  (N, 64) bf16 ----
                zc = []
                for s in range(S):
                    z = actp.tile([P, JB * U], BF16, tag=f"zc{s}")
                    for jb2 in range(JB // 2):
                        pz = psA.tile([P, 2 * U], F32, tag="psAx", bufs=2)
                        for q in range(2):
                            nc.tensor.matmul(
                                pz[:, q * U:(q + 1) * U],
                                x0cT[:, (2 * jb2 + q) * P:(2 * jb2 + q + 1) * P],
                                wc_s[s][:], start=(q == 0), stop=(q == 1))
                        nc.scalar.copy(
                            z[:, jb2 * 2 * U:(jb2 + 1) * 2 * U], pz[:])
                    zc.append(z)

                # ---- phase 2: pre_c^T ----
                cT = actp.tile([U, N], F32, tag="cT")
                for ic in range(NCH):
                    p2 = psB.tile([U, NQ], F32, tag="psB", name="p2")
                    k = 0
                    for s in range(S):
                        for jb in range(JB):
                            nc.tensor.matmul(
                                p2[:],
                                zc[s][:, jb * U:(jb + 1) * U],
                                at[s][:, jb * N + ic * NQ: jb * N + (ic + 1) * NQ],
                                start=(k == 0), stop=False)
                            k += 1
                    nc.tensor.matmul(
                        p2[:], wc0[:], x0cT[:, ic * NQ:(ic + 1) * NQ],
                        start=False, stop=True)
                    nc.scalar.activation(
                        cT[:, ic * NQ:(ic + 1) * NQ], p2[:],
                        mybir.ActivationFunctionType.Tanh, bias=bias["c"][:])

                # ---- h^T = c^T + u^T * (H^T - c^T);  back to natural ----
                hT = actp.tile([U, N], F32, tag="hT")
                nc.vector.tensor_sub(hT[:], x0T[0:U, :], cT[:])
                nc.vector.tensor_mul(hT[:], hT[:], uT[:])
                nc.vector.tensor_add(hT[:], hT[:], cT[:])
                hnat = actp.tile([P, JB * U], F32, tag="hnat")
                for jb2 in range(JB // 2):
                    ph = psA.tile([P, 2 * U], F32, tag="psAx", bufs=2)
                    for q in range(2):
                        nc.tensor.matmul(
                            ph[:, q * U:(q + 1) * U],
                            hT[:, (2 * jb2 + q) * P:(2 * jb2 + q + 1) * P],
                            id_f32[0:U, 0:U], start=(q == 0), stop=(q == 1),
                            is_transpose=True)
                    nc.scalar.copy(hnat[:, jb2 * 2 * U:(jb2 + 1) * 2 * U], ph[:])
                nc.sync.dma_start(
                    t_out.ap()[b].rearrange("(jb p u) -> p jb u", p=P, u=U),
                    hnat[:].rearrange("p (jb u) -> p jb u", u=U))

    nc.finalize()
    return nc


def _make_in_maps(inputs):
    in_maps = []
    for c in range(NCORES):
        lo, hi = c * BPC, (c + 1) * BPC
        in_maps.append({
            "inputs": np.ascontiguousarray(inputs["inputs"][lo:hi], np.float32),
            "supports": np.ascontiguousarray(inputs["supports"][lo:hi], np.float32),
            "h_prev": np.ascontiguousarray(inputs["h_prev"][lo:hi], np.float32),
            "r_kernel": np.ascontiguousarray(inputs["r_kernel"], np.float32),
            "u_kernel": np.ascontiguousarray(inputs["u_kernel"], np.float32),
            "c_kernel": np.ascontiguousarray(inputs["c_kernel"], np.float32),
            "r_bias": np.ascontiguousarray(inputs["r_bias"], np.float32),
            "u_bias": np.ascontiguousarray(inputs["u_bias"], np.float32),
            "c_bias": np.ascontiguousarray(inputs["c_bias"], np.float32),
        })
    return in_maps


def kernel(**inputs):
    nc = _COMPILED.get("nc")
    if nc is None:
        nc = _COMPILED["nc"] = _build()

    res = run_bass_kernel_spmd(nc, _make_in_maps(inputs), core_ids=list(range(NCORES)))
    out = np.concatenate([res.results[c]["out"] for c in range(NCORES)], axis=0)
    return out.astype(np.float32)
